# revision 38
# baseline (speedup 1.0000x reference)
"""Low-rank MRI CG solve on 8 trn2 cores — optimized bass kernel.

Math (see reference.py):
  A(x) = Lh M L x + lam*x, solved by 10 CG iterations for b = Lh(adjoint(y)) + lam*mo.
  With Phi(k) = sum_t conj(L_ts) L_ts' mask_t(k)  (3x3 Hermitian per k-point),
  A(x)_s = sum_c conj(s_c) .* Ginv[ sum_s' Phi_ss' .* Gf[ s_c .* x_s' ]Gf ]Ginv + lam*x_s

Distribution: coils sharded over cores (2 slots/core, zero-padded); one f32
AllReduce per iteration carries the 3 coil-sum images plus 6 dot-partial
columns; CG scalars are computed on partition 0 from those summed partials
(alpha/beta via recurrences with a direct <r,r> refresh), so the only
post-AllReduce critical path is a short scalar chain plus the p-update.
All pointwise work runs on DVE only: the GpSimd/Pool engine's 0.42-efficiency
tensor ops plus the chip power governor make any Pool activity a net loss.
FFT stages are bf16 matmuls (st1 and st2), software-pipelined one deep so the
PE never stalls on the PSUM->SBUF copies; z/r updates are deferred into idle
windows (z runs under the next AllReduce).

Pointwise complex arithmetic uses merged re/im ops: an image slab [128,1024] =
[re(h<128)|im(h<128)|re(h>=128)|im(h>=128)]; complex mult by a field F is
  A = dup(Fr) .* x ; B = sgn(Fi) .* swap(x) ; out = A +/- B
where dup(F) = [F0|F0|F1|F1], sgn(F) = [-F0|F0|-F1|F1] (host-precomputed bf16)
and swap(x) = chunk order (1,0,3,2) via a negative-stride AP view.
All these run as bf16 tensor_tensor ops (2x DVE perf mode).
"""
import sys
sys.path.insert(0, '/opt/trn_rl_repo')
import numpy as np
import ml_dtypes
import concourse.bacc as bacc
import concourse.bass as bass
import concourse.tile as tile
import concourse.mybir as mybir

F32 = mybir.dt.float32
F32R = mybir.dt.float32r
BF16 = mybir.dt.bfloat16
FP16 = mybir.dt.float16
MUL = mybir.AluOpType.mult
ADD = mybir.AluOpType.add
SUB = mybir.AluOpType.subtract
BYPASS = mybir.AluOpType.bypass
SQUARE = mybir.ActivationFunctionType.Square

N = 256
NC_CORES = 8
LAMBDA = 0.05
ITERS = 10
KCHUNK = [0, 2, 1, 3]   # slab chunk holding Ghat K-rows 128k
GSCALE = 1.0 / 16.0     # global data scale (y/b side)
ASCALE = 2.0 ** -10     # adjoint-output scale so fp16 AllReduce payloads stay in range
QSCALE = 2.0 ** -14     # extra scale on dot-partial columns (fp16 range)

# op cost estimates (us) for the DVE/Pool greedy scheduler
COST_V_TT = 1.13    # merged bf16 [128,1024] tensor_tensor on DVE
COST_G_TT = 999.0   # Pool disabled for TT: the power governor slows everything when Pool runs
COST_V_STT = 1.45   # [128,1024] scalar_tensor_tensor (no perf modes)
COST_G_STT = 4.00


# ---------------- host-side data prep ----------------

def round_f32r(a):
    b = np.ascontiguousarray(a, dtype=np.float32).view(np.uint32)
    b = ((b.astype(np.uint64) + 0x800) & 0xFFFFF000).astype(np.uint32)
    return b.view(np.float32)


def to_ptslab(img):
    """complex image [..., 256, 256] -> [..., 128, 1024] f32 ptslab."""
    re, im = np.real(img).astype(np.float32), np.imag(img).astype(np.float32)
    lead = img.shape[:-2]
    out = np.empty(lead + (128, 1024), np.float32)
    out[..., :, 0:256] = re[..., 0:128, :]
    out[..., :, 256:512] = im[..., 0:128, :]
    out[..., :, 512:768] = re[..., 128:256, :]
    out[..., :, 768:1024] = im[..., 128:256, :]
    return out


def from_ptslab(slab):
    """[128, 1024] f32 -> complex image [256,256]."""
    img = np.empty((256, 256), np.complex64)
    img[0:128] = slab[:, 0:256] + 1j * slab[:, 256:512]
    img[128:256] = slab[:, 512:768] + 1j * slab[:, 768:1024]
    return img


def dup_field(F):
    """real field [256,256] -> [128,1024] = [F0|F0|F1|F1]."""
    F0, F1 = F[0:128].astype(np.float32), F[128:256].astype(np.float32)
    return np.concatenate([F0, F0, F1, F1], axis=1)


def sgn_field(F):
    """real field [256,256] -> [128,1024] = [-F0|F0|-F1|F1]."""
    F0, F1 = F[0:128].astype(np.float32), F[128:256].astype(np.float32)
    return np.concatenate([-F0, F0, -F1, F1], axis=1)


def complexify_g(G):
    Gr, Gi = np.real(G).astype(np.float64), np.imag(G).astype(np.float64)
    top = np.concatenate([Gr, Gi], axis=1)
    bot = np.concatenate([-Gi, Gr], axis=1)
    return np.concatenate([top, bot], axis=0).astype(np.float32)


def g_to_slab(Ghat):
    out = np.empty((128, 2048), np.float32)
    for k in range(4):
        out[:, 512*k:512*(k+1)] = Ghat[128*k:128*(k+1), :]
    return out


def host_prepare(y, model_out, sens, time_basis, mask):
    """Returns per-core input maps."""
    bf = ml_dtypes.bfloat16
    yc = y[..., 0] + 1j * y[..., 1]            # [1,12,10,256,256]
    sc = sens[..., 0] + 1j * sens[..., 1]      # [1,1,10,256,256]
    mo = model_out[..., 0] + 1j * model_out[..., 1]  # [1,3,256,256]
    L = time_basis[..., 0] + 1j * time_basis[..., 1]  # [1,12,3]
    m = mask[0, :, 0]                          # [12,256,256]

    S = np.roll(np.eye(N), N // 2, axis=0)
    Fi = np.fft.ifft(np.eye(N), axis=0)
    Ff = np.fft.fft(np.eye(N), axis=0)
    Gf = S @ Fi @ S
    Gi_ = S @ Ff @ S
    gf_slab = round_f32r(g_to_slab(complexify_g(Gf)))
    gi_slab = round_f32r(g_to_slab(complexify_g(Gi_)))
    gf16 = gf_slab.astype(bf)
    gi16 = gi_slab.astype(bf)

    # Phi(k) = sum_t conj(L_ts) L_ts' m_t(k): [3,3,256,256] hermitian
    Lm = L[0]
    phi = np.einsum('ts,tu,tij->suij', np.conj(Lm), Lm, m.astype(np.complex64))
    # merged planes: dup(d0),dup(d1),dup(d2), then per pair (a,b): dup(pr),sgn(pi)
    planes = [dup_field(np.real(phi[0, 0])), dup_field(np.real(phi[1, 1])),
              dup_field(np.real(phi[2, 2]))]
    for (a, b) in ((0, 1), (0, 2), (1, 2)):
        planes.append(dup_field(np.real(phi[a, b])))
        planes.append(sgn_field(np.imag(phi[a, b])))
    phi_slab = np.concatenate(planes, axis=1).astype(bf)   # [128, 9216]

    # ytil_cs(k) = sum_t conj(L_ts) y_tc(k): [10, 3, 256, 256]
    ytil = np.einsum('ts,tcij->csij', np.conj(Lm), yc[0]) * GSCALE
    ytil_slabs = to_ptslab(ytil)                # [10, 3, 128, 1024] f32

    lmo_flat = to_ptslab(LAMBDA * GSCALE * mo[0])  # [3,128,1024]
    lmo_cat = np.concatenate([lmo_flat[i] for i in range(3)],
                             axis=1).astype(np.float16)  # [128,3072]

    assign = [(0, 1), (2, 3), (4, None), (5, None), (6, None), (7, None),
              (8, None), (9, None)]
    in_maps = []
    for core in range(NC_CORES):
        c0, c1 = assign[core]
        sensp = np.zeros((128, 4096), np.float32)   # [dup sr0|sgn si0|dup sr1|sgn si1]
        ytil_cat = np.zeros((128, 6144), np.float32)
        for slot, c in enumerate((c0, c1)):
            if c is None:
                continue
            sr = np.real(sc[0, 0, c])
            si = np.imag(sc[0, 0, c])
            sensp[:, 2048*slot:2048*slot+1024] = dup_field(sr)
            sensp[:, 2048*slot+1024:2048*slot+2048] = sgn_field(si)
            for s in range(3):
                ytil_cat[:, 3072*slot + 1024*s: 3072*slot + 1024*(s+1)] = ytil_slabs[c, s]
        in_maps.append(dict(gf16=gf16, gi16=gi16,
                            phi=phi_slab, sensp=sensp.astype(bf),
                            senspa=(sensp * ASCALE).astype(bf),
                            ytil=ytil_cat.astype(bf), lmo=lmo_cat))
    return in_maps


# ---------------- device kernel ----------------

class Sched:
    """Greedy least-finish-time assignment over DVE ('v') and Pool ('g')."""

    def __init__(self, nc):
        self.nc = nc
        self.t = {'v': 0.0, 'g': 0.0}

    def pick(self, cost_v, cost_g):
        if self.t['v'] + cost_v <= self.t['g'] + cost_g:
            self.t['v'] += cost_v
            return self.nc.vector
        self.t['g'] += cost_g
        return self.nc.gpsimd

    def dve(self, cost_v):
        self.t['v'] += cost_v
        return self.nc.vector


def build(n_iters=ITERS):
    nc = bacc.Bacc("TRN2", debug=False, num_devices=NC_CORES)
    gf16_d = nc.dram_tensor("gf16", [128, 2048], BF16, kind="ExternalInput")
    gi16_d = nc.dram_tensor("gi16", [128, 2048], BF16, kind="ExternalInput")
    phi_d = nc.dram_tensor("phi", [128, 9216], BF16, kind="ExternalInput")
    sensp_d = nc.dram_tensor("sensp", [128, 4096], BF16, kind="ExternalInput")
    senspa_d = nc.dram_tensor("senspa", [128, 4096], BF16, kind="ExternalInput")
    ytil_d = nc.dram_tensor("ytil", [128, 6144], BF16, kind="ExternalInput")
    lmo_d = nc.dram_tensor("lmo", [128, 3072], FP16, kind="ExternalInput")
    zout_d = nc.dram_tensor("zout", [128, 3072], F32, kind="ExternalOutput")
    # AllReduce tensors, one per round; fp16 payload = 3 images + 6 dot cols
    n_rounds = n_iters + 1   # setup round is index n_iters
    cc_in = [nc.dram_tensor(f"cci{j}", [128, 3078], FP16) for j in range(n_rounds)]
    cc_out = [nc.dram_tensor(f"cco{j}", [128, 3078], FP16, addr_space="Shared")
              for j in range(n_rounds)]

    with tile.TileContext(nc) as tc:
        with tc.tile_pool(name="sb", bufs=1) as pool, \
             tc.tile_pool(name="ps", bufs=2, space="PSUM") as psum:
            sched = Sched(nc)
            uid = [0]

            def nid(p):
                uid[0] += 1
                return f"{p}{uid[0]}"

            # persistent tiles
            gf16 = pool.tile([128, 2048], BF16, tag="gf16")
            gi16 = pool.tile([128, 2048], BF16, tag="gi16")
            phi = pool.tile([128, 9216], BF16, tag="phi")
            sensp = pool.tile([128, 4096], BF16, tag="sensp")
            senspa = pool.tile([128, 4096], BF16, tag="senspa")
            p16a = pool.tile([128, 3072], BF16, tag="p16a")
            p16b = pool.tile([128, 3072], BF16, tag="p16b")
            r_t = pool.tile([128, 3072], F32, tag="r")
            z_t = pool.tile([128, 3072], F32, tag="z")
            S_t = pool.tile([128, 3072], FP16, tag="S")     # AllReduce output
            App = pool.tile([128, 3072], FP16, tag="App")
            w_t = [pool.tile([128, 3072], BF16, tag=f"w{s}", name=f"w{s}")
                   for s in range(2)]   # also reused as ktl
            kap = [pool.tile([128, 3072], BF16, tag=f"kap{s}", name=f"kap{s}")
                   for s in range(2)]
            junk = pool.tile([128, 1024], BF16, tag="junk", bufs=4)
            dc16 = pool.tile([128, 6], FP16, tag="dc16")    # AR'd dot cols (xQSCALE)
            dc16r = pool.tile([128, 6], FP16, tag="dc16r")  # AR'd dot cols, received
            dc = pool.tile([128, 12], F32, tag="dc")        # 0-5 recv, 6-8 pp, 9-11 rr
            dcS = pool.tile([128, 3], F32, tag="dcS")       # <S,S> partials
            ones = pool.tile([128, 1], F32, tag="ones")
            onesr = pool.tile([1, 128], F32, tag="onesr")
            sc_t = pool.tile([1, 12], F32, tag="sc_t")      # scalar scratch (part 0)
            bc_row = pool.tile([1, 4], F32, tag="bc_row")   # to broadcast
            scall = psum.tile([128, 128], F32, tag="scall", bufs=1)
            scp = scall[0:1, 0:16]
            scp2 = scall[0:1, 32:35]
            scb = scall[:, 64:67]
            scb2 = scall[:, 96:97]

            nc.sync.dma_start(gf16[:], gf16_d[:, :])
            nc.sync.dma_start(gi16[:], gi16_d[:, :])
            nc.sync.dma_start(phi[:], phi_d[:, :])
            nc.sync.dma_start(sensp[:], sensp_d[:, :])
            nc.sync.dma_start(senspa[:], senspa_d[:, :])
            nc.vector.memset(ones[:], 1.0)
            nc.vector.memset(onesr[:], 1.0)

            # ---- views ----
            def img(tile_, i):
                return tile_[:, 1024*i:1024*(i+1)]

            def mview(ap):
                return ap.rearrange("p (h c k) -> p h c k", h=2, c=2)

            def sview(ap):
                v = ap.rearrange("p (h c k) -> p h c k", h=2, c=2)
                return v[:, :, ::-1, :]

            def phip(idx):
                return phi[:, 1024*idx:1024*(idx+1)]

            def sr_dup(slot):
                return sensp[:, 2048*slot:2048*slot+1024]

            def si_sgn(slot):
                return sensp[:, 2048*slot+1024:2048*slot+2048]

            def sr_dup_a(slot):
                return senspa[:, 2048*slot:2048*slot+1024]

            def si_sgn_a(slot):
                return senspa[:, 2048*slot+1024:2048*slot+2048]

            # ---- merged complex ops ----
            def cmult(eng, out, plane_dup, plane_sgn, x, sign):
                """out = (complex field) * x via A +/- B on one engine."""
                A = pool.tile([128, 1024], BF16, tag="tmp", bufs=8, name=nid("tA"))
                B = pool.tile([128, 1024], BF16, tag="tmp", bufs=8, name=nid("tB"))
                eng.tensor_tensor(A[:], plane_dup, x, MUL)
                eng.tensor_tensor(mview(B[:]), mview(plane_sgn), sview(x), MUL)
                eng.tensor_tensor(out, A[:], B[:], sign)

            def cmult_acc(eng, out, plane_dup, plane_sgn, x, sign):
                """out += dup*x ; out (+/-)= sgn*swap(x)."""
                A = pool.tile([128, 1024], BF16, tag="tmp", bufs=8, name=nid("tA"))
                B = pool.tile([128, 1024], BF16, tag="tmp", bufs=8, name=nid("tB"))
                eng.tensor_tensor(A[:], plane_dup, x, MUL)
                eng.tensor_tensor(mview(B[:]), mview(plane_sgn), sview(x), MUL)
                eng.tensor_tensor(out, out, A[:], ADD)
                eng.tensor_tensor(out, out, B[:], sign)

            # ---- matmul machinery ----
            def mm_chain(lhsT_slab, g_sb, out_psums):
                for mtile in range(2):
                    pt = out_psums[mtile]
                    for k in range(4):
                        ch = KCHUNK[k]
                        nc.tensor.matmul(
                            pt[:],
                            lhsT_slab[:, 256*ch + 128*mtile: 256*ch + 128*(mtile+1)],
                            g_sb[:, 512*k:512*(k+1)],
                            start=(k == 0), stop=(k == 3))

            def fft2_st1(src_slab, g1):
                """stage 1 matmuls + one merged Act copy to a fresh mid tile."""
                pA = psum.tile([128, 1024], F32, tag="mmA", bufs=2, name=nid("pA"))
                mm_chain(src_slab, g1, [pA[:, 0:512], pA[:, 512:1024]])
                m_t = pool.tile([128, 1024], BF16, tag="mid", bufs=4, name=nid("mid"))
                nc.scalar.copy(m_t[:], pA[:])
                return m_t

            def fft2_st2(m_t, g2, dst_slab):
                pB = psum.tile([128, 1024], F32, tag="mmB", bufs=1, name=nid("pB"))
                mm_chain(m_t[:], g2, [pB[:, 0:512], pB[:, 512:1024]])
                nc.scalar.copy(dst_slab, pB[:])



            def fft2_pipe(jobs):
                """jobs: list of (src, g1, g2, dst, post_fn|None). Software-
                pipelined one deep so the PE never waits on mid copies."""
                mids = [None] * len(jobs)
                for j, (src, g1, _g2, _dst, _post) in enumerate(jobs):
                    mids[j] = fft2_st1(src, g1)
                    if j >= 1:
                        src2, _, g2, dst, post = jobs[j-1]
                        fft2_st2(mids[j-1], g2, dst)
                        if post is not None:
                            post()
                j = len(jobs) - 1
                _, _, g2, dst, post = jobs[j]
                fft2_st2(mids[j], g2, dst)
                if post is not None:
                    post()

            # ---- pipeline pieces ----
            def sens_mult(slot, src16):
                """w[slot][i] = sens[slot] * src16[i] for i in 0..2."""
                for i in range(3):
                    eng = sched.pick(3*COST_V_TT, 3*COST_G_TT)
                    cmult(eng, img(w_t[slot], i), sr_dup(slot), si_sgn(slot),
                          img(src16, i), ADD)

            def phi_diag(slot):
                """ktl[slot][s] = D_s * kap[slot][s] for all 3 images at once."""
                eng = sched.pick(3*COST_V_TT, 3*COST_G_TT)
                eng.tensor_tensor(w_t[slot][:], phi[:, 0:3072], kap[slot][:], MUL)

            def phi_out(slot, s):
                """ktl[slot][s] += off-diagonal Phi terms (ktl aliases w)."""
                eng = sched.pick(8*COST_V_TT, 8*COST_G_TT)
                out = img(w_t[slot], s)
                pair_idx = {(0, 1): 3, (0, 2): 5, (1, 2): 7}
                for a in range(3):
                    if a == s:
                        continue
                    lo, hi = min(s, a), max(s, a)
                    pd = phip(pair_idx[(lo, hi)])
                    ps_ = phip(pair_idx[(lo, hi)] + 1)
                    # s < a: ktl_s += P_sa * k_a ; s > a: ktl_s += conj(P_as) * k_a
                    sign = ADD if s < a else SUB
                    cmult_acc(eng, out, pd, ps_, img(kap[slot], a), sign)

            def adjoint(slot, i, zslab):
                """App[i] (+)= ASCALE * conj(sens[slot]) * zslab."""
                if slot == 0:
                    eng = sched.pick(3*COST_V_TT, 3*COST_G_TT)
                    cmult(eng, img(App, i), sr_dup_a(slot), si_sgn_a(slot),
                          zslab, SUB)
                else:
                    eng = sched.pick(4*COST_V_TT, 4*COST_G_TT)
                    cmult_acc(eng, img(App, i), sr_dup_a(slot), si_sgn_a(slot),
                              zslab, SUB)

            def emit_dots(i, p16):
                """Local dot partials vs App[i] -> dc16 cols (rA: 0-2, pA: 3-5),
                pre-scaled by QSCALE so the fp16 partials cannot overflow."""
                eng = sched.dve(2*COST_V_STT)
                eng.scalar_tensor_tensor(
                    junk[:], img(App, i), QSCALE, img(r_t, i), MUL, MUL,
                    accum_out=dc16[:, i:i+1])
                eng.scalar_tensor_tensor(
                    junk[:], img(App, i), QSCALE, img(p16, i), MUL, MUL,
                    accum_out=dc16[:, 3+i:4+i])

            def emit_send(rnd, i):
                """Stage image i of App (+ its dot cols) into the AR input."""
                nc.sync.dma_start(cc_in[rnd][:, 1024*i:1024*(i+1)], img(App, i))

            def emit_ar(rnd, during_fn=None):
                """One fp16 AllReduce per round: 3 App images + 6 dot cols.
                during_fn emits work that overlaps the collective."""
                nc.sync.dma_start(cc_in[rnd][:, 3072:3078], dc16[:, 0:6])
                nc.gpsimd.collective_compute(
                    "AllReduce", ADD, replica_groups=[list(range(NC_CORES))],
                    ins=[cc_in[rnd].ap().opt()], outs=[cc_out[rnd].ap().opt()])
                if during_fn is not None:
                    during_fn()
                nc.sync.dma_start(S_t[:], cc_out[rnd][:, 0:3072])
                nc.sync.dma_start(dc16r[:, 0:6], cc_out[rnd][:, 3072:3078])

            # ================= setup: b = AR(adjoint(Ginv ytil)) + lmo ========
            # ytil (bf16) loaded directly as "ktl"; run the inverse+adjoint path.
            nc.sync.dma_start(w_t[0][:], ytil_d[:, 0:3072])
            nc.sync.dma_start(w_t[1][:], ytil_d[:, 3072:6144])
            lmo_sb = pool.tile([128, 3072], FP16, tag="lmo_sb")
            nc.sync.dma_start(lmo_sb[:], lmo_d[:, :])
            setup_jobs = []
            for i in range(3):
                for slot in range(2):
                    def mk_post(sl, ii):
                        zr = pool.tile([128, 1024], BF16, tag="zres2",
                                       bufs=4, name=nid("zr"))

                        def post():
                            adjoint(sl, ii, zr[:])
                            if sl == 1:
                                emit_send(n_iters, ii)
                        return zr, post
                    zr, post = mk_post(slot, i)
                    setup_jobs.append((img(w_t[slot], i), gi16, gi16, zr[:], post))
            fft2_pipe(setup_jobs)
            nc.vector.memset(dc16[:, 0:6], 0.0)
            emit_ar(n_iters)
            # r = S/ASCALE + lmo ; p16a = bf16(r); z = 0
            nc.vector.scalar_tensor_tensor(r_t[:], S_t[:], 1.0 / ASCALE,
                                           lmo_sb[:], MUL, ADD)
            nc.scalar.copy(p16a[:], r_t[:])
            nc.vector.memset(z_t[:], 0.0)
            # rr cols (9-11) from r
            for i in range(3):
                nc.scalar.activation(junk[:], img(r_t, i), SQUARE,
                                     accum_out=dc[:, 9+i:10+i])
            nc.vector.memset(dc[:, 6:9], 0.0)

            # ================= CG iterations ========
            # r is updated without the O(5e-5-relative) lambda correction:
            # r_new = r - alpha*S.  p still follows the exact reference
            # recursion because bmal = beta - alpha*lambda folds it back in:
            # p_new = (r - alpha*S) + bmal*p = r_ref_new + beta*p.
            zpend = [None]
            pend_tp = [None]

            def emit_tp(i):
                """Pending r_new_i / p_new_i from the previous tail."""
                if pend_tp[0] is None:
                    return
                p16_o, p16_n, nal_, bmal_ = pend_tp[0]
                nc.vector.scalar_tensor_tensor(
                    img(r_t, i), img(S_t, i), nal_, img(r_t, i), MUL, ADD)
                nc.vector.scalar_tensor_tensor(
                    img(p16_n, i), img(p16_o, i), bmal_, img(r_t, i), MUL, ADD)
                sched.t['v'] += 2.9
                if i == 2:
                    pend_tp[0] = None

            def emit_z():
                """z += alpha*p_old — emitted under the AllReduce window."""
                if zpend[0] is None:
                    return
                p16_old, alf_ = zpend[0]
                nc.vector.scalar_tensor_tensor(z_t[:], p16_old[:], alf_, z_t[:],
                                               MUL, ADD)
                zpend[0] = None

            for it in range(n_iters):
                p16 = p16a if it % 2 == 0 else p16b
                p16_new = p16b if it % 2 == 0 else p16a
                sched.t = {'v': 0.0, 'g': 0.0}

                # --- phase A: sens mult + forward ffts, pipelined ---
                for i in range(3):
                    emit_tp(i)
                    eng = sched.pick(3*COST_V_TT, 3*COST_G_TT)
                    cmult(eng, img(w_t[0], i), sr_dup(0), si_sgn(0),
                          img(p16, i), ADD)
                sens_mult(1, p16)
                # <r,r> partials (cols 9-11): direct, r from last tail
                for i in range(3):
                    nc.scalar.activation(junk[:], img(r_t, i), SQUARE,
                                         accum_out=dc[:, 9+i:10+i])

                # One combined 12-job pipeline keeps the PE MM stream dense
                # (no long PE-idle gap mid-iteration, so HAM stays warm):
                # fwd(s0) -> [phi(0)] -> fwd(s1) -> [phi(1)] -> inv(s0) -> inv(s1)
                def mk_phi(slot):
                    def post():
                        phi_diag(slot)
                        for s in range(3):
                            phi_out(slot, s)
                    return post

                jobs = []
                for slot in range(2):
                    for i in range(3):
                        post = mk_phi(slot) if i == 2 else None
                        jobs.append((img(w_t[slot], i), gf16, gf16,
                                     img(kap[slot], i), post))
                for slot in range(2):
                    for i in range(3):
                        def mk_post(sl, ii, rnd):
                            zr = pool.tile([128, 1024], BF16, tag="zres2",
                                           bufs=4, name=nid("zr"))

                            def post():
                                adjoint(sl, ii, zr[:])
                                if sl == 1:
                                    emit_dots(ii, p16)
                                    emit_send(rnd, ii)
                            return zr, post
                        zr, post = mk_post(slot, i, it)
                        jobs.append((img(w_t[slot], i), gi16, gi16, zr[:], post))
                fft2_pipe(jobs)
                emit_ar(it, during_fn=emit_z)

                # --- tail: alpha chain first (does not wait on <S,S>) ---
                nc.vector.tensor_copy(dc[:, 0:6], dc16r[:, 0:6])
                nc.tensor.matmul(scp[:, 0:12], ones[:], dc[:, 0:12],
                                 start=True, stop=True)
                nc.vector.tensor_reduce(
                    sc_t[:, 0:4], scp[:, 0:12].rearrange("p (g c) -> p g c", g=4),
                    mybir.AxisListType.X, ADD)
                # cols 0-1 (rA, pA) came through the fp16 AR with QSCALE*ASCALE
                nc.vector.tensor_scalar_mul(sc_t[:, 0:2], sc_t[:, 0:2],
                                            1.0 / (QSCALE * ASCALE))
                rA, pA, pp, rr = (sc_t[:, k:k+1] for k in range(4))
                t1, t2, t3 = sc_t[:, 5:6], sc_t[:, 6:7], sc_t[:, 7:8]
                al, SS_ = sc_t[:, 8:9], sc_t[:, 9:10]
                # pAp = pA + lam*pp ; alpha = rr/pAp
                nc.vector.scalar_tensor_tensor(t1, pp, LAMBDA, pA, MUL, ADD)
                nc.vector.reciprocal(t1, t1)
                nc.vector.tensor_tensor(al, rr, t1, MUL)          # alpha
                # broadcast alpha-derived scalars immediately (t/r_new/z use these)
                # nal multiplies the ASCALE-scaled S, so fold in 1/ASCALE
                nc.vector.tensor_scalar_mul(bc_row[:, 0:1], al, -1.0 / ASCALE)
                nc.vector.tensor_copy(bc_row[:, 1:2], al)
                nc.tensor.matmul(scb[:, 0:2], onesr[:], bc_row[:, 0:2],
                                 start=True, stop=True)
                nal, alf = scb[:, 0:1], scb[:, 1:2]
                # --- beta chain: needs <S,S> squares ---
                for i in range(3):
                    nc.scalar.activation(junk[:], img(S_t, i), SQUARE,
                                         accum_out=dcS[:, i:i+1])
                nc.tensor.matmul(scp2[:], ones[:], dcS[:], start=True, stop=True)
                nc.vector.tensor_reduce(
                    SS_, scp2[:, 0:3].rearrange("p (g c) -> p g c", g=1),
                    mybir.AxisListType.X, ADD)
                # squares were of the ASCALE-scaled S
                nc.vector.tensor_scalar_mul(SS_, SS_, 1.0 / (ASCALE * ASCALE))
                # rs_new = rr + alpha*(alpha*SS - 2*rA)  (r carries no lambda)
                nc.vector.tensor_tensor(t2, SS_, al, MUL)
                nc.vector.scalar_tensor_tensor(t2, rA, -2.0, t2, MUL, ADD)
                nc.vector.tensor_tensor(t2, t2, al, MUL)
                nc.vector.tensor_tensor(t2, rr, t2, ADD)          # rs_new
                # beta = rs_new / rr ; bmal = beta - alpha*lam
                nc.vector.reciprocal(t3, rr)
                nc.vector.tensor_tensor(t3, t2, t3, MUL)          # beta
                nc.vector.scalar_tensor_tensor(bc_row[:, 3:4], al, -LAMBDA, t3,
                                               MUL, ADD)
                nc.tensor.matmul(scb2[:], onesr[:], bc_row[:, 3:4], start=True,
                                 stop=True)
                bmal = scb2[:, 0:1]
                # r/p_new interleave into the next iteration's phase A;
                # z is deferred into the AllReduce window
                pend_tp[0] = (p16, p16_new, nal, bmal)
                zpend[0] = (p16, alf)

            emit_z()
            nc.sync.dma_start(zout_d[:, :], z_t[:])
    nc.compile()
    return nc


# ---------------- full kernel entry ----------------

_cached = {}
LAST_EXEC_NS = None


def _install_ntff_shim():
    import types
    name = "antenv.axon_hooks"
    if name in sys.modules:
        return True
    mod = types.ModuleType(name)
    mod._hook = None
    mod.set_axon_ntff_profile_hook = lambda h: setattr(mod, "_hook", h)
    mod.get_axon_ntff_profile_hook = lambda: mod._hook
    sys.modules[name] = mod
    try:
        import antenv
        antenv.axon_hooks = mod
    except ImportError:
        pkg = types.ModuleType("antenv")
        pkg.axon_hooks = mod
        pkg.__path__ = []
        sys.modules["antenv"] = pkg
    try:
        from trn_agent_boot.trn_boot import _ntff_profile_via_ctypes
        hook = _ntff_profile_via_ctypes('/opt/axon/libaxon_pjrt.so')
        if hook is not None:
            mod._hook = hook
            return True
    except Exception:
        pass
    return False


def kernel(y, model_out, sens, time_basis, mask):
    global LAST_EXEC_NS
    import os
    from concourse.bass_utils import run_bass_kernel_spmd
    trace = os.environ.get("KERNEL_TRACE", "0") == "1" and _install_ntff_shim()
    in_maps = host_prepare(np.asarray(y), np.asarray(model_out), np.asarray(sens),
                           np.asarray(time_basis), np.asarray(mask))
    if "nc" not in _cached:
        _cached["nc"] = build(ITERS)
    nc = _cached["nc"]
    res = run_bass_kernel_spmd(nc, in_maps, list(range(NC_CORES)), trace=trace)
    LAST_EXEC_NS = res.exec_time_ns
    z = res.results[0]["zout"] * (1.0 / GSCALE)
    out = np.empty((1, 3, 256, 256, 2), np.float32)
    for s in range(3):
        imgc = from_ptslab(z[:, 1024*s:1024*(s+1)])
        out[0, s, :, :, 0] = np.real(imgc)
        out[0, s, :, :, 1] = np.imag(imgc)
    return out

